# revision 22
# baseline (speedup 1.0000x reference)
"""Causal attention (LN -> QKV -> 16-head causal attn -> out-proj) on 8 TRN2 cores.

Sharding: core c = (batch b=c//4, head-group g=c%4); each core does its batch's
LayerNorm + 4 heads of QKV/attention/out-proj; host sums the 4 column-split
out-proj partials per batch.

Single fused pipeline per core:
  - x arrives bf16 per seq-quarter (st); LN fully on DVE (bn_stats + Quake
    rsqrt via int32 bit tricks) so ScalarE runs ONLY exp (ACT table loaded
    once, preloaded by a dummy activation).
  - xn transposed to xnT (chunk order d = 128*kb+p): PE identity-matmuls for
    st0 (startup latency; evacs split Scalar/DVE), DMA XBAR for st1-st3.
  - attention as 8 (pr, qt) groups of 512 queries x 2 heads; S^T for both
    heads in ONE [128,1024] psum tile (two 64-contraction matmuls, operands in
    per-head [64,*] Q/K tiles at partitions 0-63) -> ONE exp per (pr,qt,kb).
    The NEXT group's first S is prefetched inside the current group's last
    iteration so the exp stream never waits on group turnover.
  - V tiles carry 64 ones-columns so PV also lands 64 copies of the softmax
    denominator in psum rows 64:127: normalize = DVE copy + fast reciprocal +
    one multiply. Causal diagonal handled by a broadcast tri-mult on GpSimd
    (whose queue carries nothing else, so exp-waits cannot block other work).
  - QKV for st2/st3 and the out-projection are emitted as small "filler"
    units inside the attention kb loops to keep the PE busy during exp.
  PSUM: S [128,1024]x2bufs (4 banks) + psO 2x[128,512] (2) + general-purpose
  [128,1024] (2) = 8 banks exactly.
"""

import numpy as np
import ml_dtypes

import concourse.bass as bass
import concourse.mybir as mybir
import concourse.tile as tile
from concourse import bacc
from concourse.bass_utils import run_bass_kernel_spmd
from concourse.masks import make_identity

B, N, DIM, HEADS, DIM_HEAD = 2, 2048, 1024, 16, 64
INNER = HEADS * DIM_HEAD
H_LOC = 4
N_CORES = 8
P = 128
NB = N // P
KB = DIM // P
QT = 512
SCALE = DIM_HEAD ** -0.5
LN_EPS = 1e-5
MAGIC = 0x5F3759DF

F32 = mybir.dt.float32
BF16 = mybir.dt.bfloat16
I32 = mybir.dt.int32
AF = mybir.ActivationFunctionType
ALU = mybir.AluOpType


def build_nc(no_bias=False):
    from contextlib import ExitStack

    nc = bacc.Bacc(None, target_bir_lowering=False, debug=False)

    x_d = nc.dram_tensor("x", [N, DIM], BF16, kind="ExternalInput")
    wq_d = nc.dram_tensor("wq", [P, KB, 2 * P], BF16, kind="ExternalInput")
    wk_d = nc.dram_tensor("wk", [P, KB, 2 * P], BF16, kind="ExternalInput")
    wv_d = nc.dram_tensor("wv", [P, KB, 2 * P], BF16, kind="ExternalInput")
    wo_d = nc.dram_tensor("wo", [P, 2, DIM], BF16, kind="ExternalInput")
    bq_d = nc.dram_tensor("bq", [P, 2], F32, kind="ExternalInput")
    bk_d = nc.dram_tensor("bk", [P, 2], F32, kind="ExternalInput")
    bv_d = nc.dram_tensor("bv", [1, H_LOC, DIM_HEAD], F32, kind="ExternalInput")
    tri_d = nc.dram_tensor("tri", [P, P], BF16, kind="ExternalInput")
    out_d = nc.dram_tensor("out", [N, DIM], BF16, kind="ExternalOutput")

    with tile.TileContext(nc) as tc:
        ctx = ExitStack()
        with ctx:
            const = ctx.enter_context(tc.tile_pool(name="const", bufs=1))
            persist = ctx.enter_context(tc.tile_pool(name="persist", bufs=1))
            xnpool = ctx.enter_context(tc.tile_pool(name="xnpool", bufs=4))
            statp = ctx.enter_context(tc.tile_pool(name="statp", bufs=2))
            expp = ctx.enter_context(tc.tile_pool(name="expp", bufs=4))
            dsb = ctx.enter_context(tc.tile_pool(name="dsb", bufs=2))
            stage = ctx.enter_context(tc.tile_pool(name="stage", bufs=3))
            psS = ctx.enter_context(tc.tile_pool(name="psS", bufs=2, space="PSUM"))
            psO = ctx.enter_context(tc.tile_pool(name="psO", bufs=1, space="PSUM"))
            psG = ctx.enter_context(tc.tile_pool(name="psG", bufs=1, space="PSUM"))

            # exp ACT-table preload
            dmy_f = const.tile([P, 1], F32, tag="dmy_f", name="dmy_f")
            nc.vector.memset(dmy_f, 0.0)
            dmy_b = const.tile([P, 1], BF16, tag="dmy_b", name="dmy_b")
            nc.scalar.activation(dmy_b[:], dmy_f[:], AF.Exp)
            ident = const.tile([P, P], BF16, tag="ident", name="ident")
            make_identity(nc, ident)

            # ---- input DMAs: x(st0) first, weights interleaved ----
            x_st = [persist.tile([P, 4, DIM], BF16, tag=f"xs{st}", name=f"xs{st}")
                    for st in range(4)]
            for j in range(4):
                nc.sync.dma_start(x_st[0][:, j, :], x_d[j * P:(j + 1) * P, :])
            wq_sb = persist.tile([P, KB, 2 * P], BF16, tag="wq", name="wq_sb")
            nc.sync.dma_start(wq_sb[:], wq_d[:])
            wk_sb = persist.tile([P, KB, 2 * P], BF16, tag="wk", name="wk_sb")
            nc.sync.dma_start(wk_sb[:], wk_d[:])
            nc.sync.dma_start(
                x_st[1][:], x_d[512:1024, :].rearrange("(s p) d -> p s d", p=P))
            wv_sb = persist.tile([P, KB, 2 * P], BF16, tag="wv", name="wv_sb")
            nc.sync.dma_start(wv_sb[:], wv_d[:])
            nc.sync.dma_start(
                x_st[2][:], x_d[1024:1536, :].rearrange("(s p) d -> p s d", p=P))
            nc.sync.dma_start(
                x_st[3][:], x_d[1536:2048, :].rearrange("(s p) d -> p s d", p=P))
            wo_sb = persist.tile([P, 2, DIM], BF16, tag="wo", name="wo_sb")
            nc.sync.dma_start(wo_sb[:], wo_d[:])
            bq_sb = const.tile([P, 2], F32, tag="bq", name="bq_sb")
            nc.sync.dma_start(bq_sb[:], bq_d[:])
            bk_sb = const.tile([P, 2], F32, tag="bk", name="bk_sb")
            nc.sync.dma_start(bk_sb[:], bk_d[:])
            bv_sb = const.tile([P, H_LOC, DIM_HEAD], F32, tag="bv", name="bv_sb")
            nc.sync.dma_start(bv_sb[:], bv_d[:].to_broadcast((P, H_LOC, DIM_HEAD)))
            tri_t = const.tile([P, P], BF16, tag="tri", name="tri_t")
            nc.sync.dma_start(tri_t[:], tri_d[:])
            magic_t = const.tile([P, 4], I32, tag="magic", name="magic_t")
            nc.vector.memset(magic_t, MAGIC)

            xnT = [persist.tile([P, KB, 4 * P], BF16, tag=f"xnT{st}",
                                name=f"xnT{st}") for st in range(4)]
            QTh = [[persist.tile([DIM_HEAD, N], BF16, tag=f"qt{pr}{hh}",
                                 name=f"qt{pr}{hh}") for hh in range(2)]
                   for pr in range(2)]
            KTh = [[persist.tile([DIM_HEAD, N], BF16, tag=f"kt{pr}{hh}",
                                 name=f"kt{pr}{hh}") for hh in range(2)]
                   for pr in range(2)]
            Vt = persist.tile([P, NB, H_LOC, P], BF16, tag="v", name="Vt")
            nc.gpsimd.memset(Vt[:], 1.0)
            outT = [persist.tile([P, N], BF16, tag=f"outT{pr}", name=f"outT{pr}")
                    for pr in range(2)]

            # ---- LN / QKV building blocks ----
            # psum allocator for accumulation chains: during startup rotate
            # across psG + the two (still unused) psS slots so transpose/V/QK
            # chains overlap their evacuations; attention-era fillers use psG.
            _ps_state = {"rr": 0, "startup": True}

            def mkps(name):
                if _ps_state["startup"]:
                    _ps_state["rr"] = (_ps_state["rr"] + 1) % 3
                    if _ps_state["rr"]:
                        return psS.tile([P, 2 * QT], F32, tag="sps",
                                        name=f"ps_{name}")
                return psG.tile([P, KB * P], F32, tag="gp", name=f"ps_{name}")

            stp_t = {}

            def u_stats(st, j):
                if st not in stp_t:
                    stp_t[st] = statp.tile([P, 4, 2], F32, tag="stp",
                                           name=f"stp{st}")
                st6 = statp.tile([P, 2, 6], F32, tag="st6", name=f"st6_{st}_{j}")
                x3 = x_st[st][:, j, :].rearrange("p (a f) -> p a f", a=2)
                for a in range(2):
                    nc.vector.bn_stats(st6[:, a, :], x3[:, a, :])
                nc.vector.bn_aggr(stp_t[st][:, j, :], st6[:])

            rs_t = {}

            def u_rsqrt(st, j0, n):
                stp = stp_t[st]
                veps = statp.tile([P, 4], F32, tag="veps", name=f"ve{st}{j0}")
                nc.vector.tensor_scalar_add(
                    veps[:, 0:n], stp[:, j0:j0 + n, 1], LN_EPS)
                iv = statp.tile([P, 4], I32, tag="iv", name=f"iv{st}{j0}")
                nc.vector.tensor_scalar(
                    iv[:, 0:n], veps[:, 0:n].bitcast(I32), 1, None,
                    ALU.logical_shift_right)
                y0i = statp.tile([P, 4], I32, tag="y0i", name=f"y0{st}{j0}")
                nc.vector.tensor_tensor(
                    y0i[:, 0:n], magic_t[:, 0:n], iv[:, 0:n], ALU.subtract)
                t1 = statp.tile([P, 4], F32, tag="t1", name=f"t1{st}{j0}")
                if (st, 0) not in rs_t:
                    rs_t[(st, 0)] = statp.tile([P, 4], F32, tag="rstd",
                                               name=f"rstd{st}")
                    rs_t[(st, 1)] = statp.tile([P, 4], F32, tag="nmrs",
                                               name=f"nmrs{st}")
                rstd = rs_t[(st, 0)][:, j0:j0 + n]
                nmrs = rs_t[(st, 1)][:, j0:j0 + n]
                for it in range(2):
                    src = y0i[:, 0:n].bitcast(F32) if it == 0 else rstd
                    nc.vector.tensor_tensor(t1[:, 0:n], src, src, ALU.mult)
                    nc.vector.tensor_tensor(
                        t1[:, 0:n], t1[:, 0:n], veps[:, 0:n], ALU.mult)
                    nc.vector.tensor_scalar(
                        t1[:, 0:n], t1[:, 0:n], -0.5, 1.5, ALU.mult, ALU.add)
                    nc.vector.tensor_tensor(rstd, src, t1[:, 0:n], ALU.mult)
                nc.vector.tensor_tensor(
                    nmrs, stp[:, j0:j0 + n, 0], rstd, ALU.mult)
                nc.vector.tensor_scalar_mul(nmrs, nmrs, -1.0)

            def u_xnT(st, j):
                sb = st * 4 + j
                xn = xnpool.tile([P, DIM], BF16, tag="xn", name=f"xn{sb}")
                nc.vector.tensor_scalar(
                    xn[:], x_st[st][:, j, :],
                    rs_t[(st, 0)][:, j:j + 1], rs_t[(st, 1)][:, j:j + 1],
                    ALU.mult, ALU.add)
                if st == 0:
                    gp = mkps(f"gpt{sb}")
                    for kb in range(KB):
                        nc.tensor.matmul(
                            gp[:, kb * P:(kb + 1) * P],
                            xn[:, kb * P:(kb + 1) * P],
                            ident[:],
                            start=True, stop=True)
                    dst = xnT[st][:, :, j * P:(j + 1) * P]
                    srcv = gp[:].rearrange("p (kb s) -> p kb s", kb=KB)
                    if j % 2 == 0:
                        nc.scalar.copy(dst, srcv)
                    else:
                        nc.vector.tensor_copy(dst, srcv)
                else:
                    nc.sync.dma_start(
                        xnT[st][:, :, j * P:(j + 1) * P], xn[:], transpose=True)

            def u_v(st, j):
                sb = st * 4 + j
                gp = mkps(f"gpv{sb}")
                for kb in range(KB):
                    nc.tensor.matmul(
                        gp[:, 0:2 * P],
                        xnT[st][:, kb, j * P:(j + 1) * P],
                        wv_sb[:, kb, :],
                        start=(kb == 0), stop=(kb == KB - 1))
                nc.vector.tensor_tensor(
                    Vt[:, sb, :, 0:DIM_HEAD],
                    gp[:, 0:2 * P].rearrange("p (h d) -> p h d", h=H_LOC),
                    bv_sb[:],
                    ALU.add)

            def u_qk(st, wt, pr):
                w_sb, bias, dst = (
                    (wq_sb, bq_sb, QTh) if wt == 0 else (wk_sb, bk_sb, KTh))
                gp = mkps(f"gpqk{st}{wt}{pr}")
                for kb in range(KB):
                    nc.tensor.matmul(
                        gp[:, 0:512],
                        w_sb[:, kb, pr * P:(pr + 1) * P],
                        xnT[st][:, kb, :],
                        start=(kb == 0), stop=(kb == KB - 1))
                for hh in range(2):
                    d_ap = dst[pr][hh][:, st * 512:(st + 1) * 512]
                    s_ap = gp[hh * DIM_HEAD:(hh + 1) * DIM_HEAD, 0:512]
                    if no_bias and st <= 1:
                        # ScalarE COPY needs no ACT table; its queue has slack
                        # in the small-exp era where st1 QKV must land, while
                        # DVE is congested there.
                        nc.scalar.copy(d_ap, s_ap)
                    else:
                        nc.vector.tensor_scalar_add(
                            d_ap, s_ap,
                            bias[hh * DIM_HEAD:(hh + 1) * DIM_HEAD, pr:pr + 1])

            def u_outproj(qb):
                gp = mkps(f"gpo{qb}")
                for nt in range(2):
                    for pr in range(2):
                        nc.tensor.matmul(
                            gp[:, nt * 512:(nt + 1) * 512],
                            outT[pr][:, qb * P:(qb + 1) * P],
                            wo_sb[:, pr, nt * 512:(nt + 1) * 512],
                            start=(pr == 0), stop=(pr == 1))
                so = stage.tile([P, DIM], BF16, tag="so", name=f"so{qb}")
                if qb >= 12 and qb % 2:
                    nc.scalar.copy(so[:], gp[:])
                else:
                    nc.vector.tensor_copy(so[:], gp[:])
                nc.sync.dma_start(out_d[qb * P:(qb + 1) * P, :], so[:])

            # ---- straight phase A for st0 (latency-tuned) and st1-LN ----
            for pair in range(2):
                for j in (2 * pair, 2 * pair + 1):
                    u_stats(0, j)
                u_rsqrt(0, 2 * pair, 2)
                for j in (2 * pair, 2 * pair + 1):
                    u_xnT(0, j)
                    u_stats(1, 2 * pair + (j % 2))
            for wt in range(2):
                for pr in range(2):
                    u_qk(0, wt, pr)
            u_v(0, 0)
            u_rsqrt(1, 0, 4)
            for j in range(4):
                u_xnT(1, j)
            _ps_state["startup"] = False

            # ---- fillers ----
            fillers = []

            def pop_filler():
                if fillers:
                    fillers.pop(0)[1]()

            def drain_required(gi):
                while fillers and fillers[0][0] <= gi:
                    fillers.pop(0)[1]()

            def add_ln_units(st, deadline):
                for j in range(4):
                    fillers.append(
                        (deadline, (lambda s, j: lambda: u_stats(s, j))(st, j)))
                fillers.append(
                    (deadline, (lambda s: lambda: u_rsqrt(s, 0, 4))(st)))
                for j in range(4):
                    fillers.append(
                        (deadline, (lambda s, j: lambda: u_xnT(s, j))(st, j)))

            def add_vqk_units(st, deadline):
                for j in range(4):
                    fillers.append(
                        (deadline, (lambda s, j: lambda: u_v(s, j))(st, j)))
                for wt in range(2):
                    for pr in range(2):
                        fillers.append(
                            (deadline,
                             (lambda s, w, p_: lambda: u_qk(s, w, p_))(st, wt, pr)))

            def add_op_units(q0):
                for qb in range(q0, q0 + 4):
                    fillers.append((99, (lambda q: lambda: u_outproj(q))(qb)))

            for j in range(1, 4):
                fillers.append((0, (lambda jj: lambda: u_v(0, jj))(j)))
            for j in range(4):
                fillers.append((2, (lambda jj: lambda: u_v(1, jj))(j)))
            for wt in range(2):
                for pr in range(2):
                    fillers.append(
                        (2, (lambda w, p_: lambda: u_qk(1, w, p_))(wt, pr)))

            # ---- fused attention pipeline ----
            groups = [(pr, qt) for qt in range(4) for pr in range(2)]

            def s_mm(pr, qt, kb):
                voff = max(0, (kb - 4 * qt) * P)
                sps = psS.tile([P, 2 * QT], F32, tag="sps",
                               name=f"sps{pr}_{qt}_{kb}")
                for hh in range(2):
                    nc.tensor.matmul(
                        sps[:, hh * QT + voff:(hh + 1) * QT],
                        KTh[pr][hh][:, kb * P:(kb + 1) * P],
                        QTh[pr][hh][:, qt * QT + voff:(qt + 1) * QT],
                        start=True, stop=True)
                return sps, voff

            sps_next = s_mm(0, 0, 0)
            for gi, (pr, qt) in enumerate(groups):
                last_kb = 4 * qt + 3
                po = [psO.tile([P, QT], F32, tag=f"po{hh}", name=f"po{hh}_{gi}")
                      for hh in range(2)]
                for kb in range(last_kb + 1):
                    sps, voff = sps_next
                    ex = expp.tile([P, 2 * QT], BF16, tag="ex",
                                   name=f"ex{gi}_{kb}")
                    ex3 = ex[:].rearrange("p (h q) -> p h q", h=2)
                    sp3 = sps[:].rearrange("p (h q) -> p h q", h=2)
                    nc.scalar.activation(
                        ex3[:, :, voff:QT], sp3[:, :, voff:QT], AF.Exp)
                    if kb < last_kb:
                        sps_next = s_mm(pr, qt, kb + 1)
                    elif gi + 1 < len(groups):
                        drain_required(gi + 1)
                        npr, nqt = groups[gi + 1]
                        sps_next = s_mm(npr, nqt, 0)
                    if kb >= 4 * qt:
                        tri_b = tri_t[:].rearrange(
                            "p (o q) -> p o q", o=1).to_broadcast((P, 2, P))
                        nc.gpsimd.tensor_tensor(
                            ex3[:, :, voff:voff + P],
                            ex3[:, :, voff:voff + P],
                            tri_b,
                            ALU.mult)
                    for hh in range(2):
                        nc.tensor.matmul(
                            po[hh][:, voff:QT],
                            Vt[:, kb, 2 * pr + hh, :],
                            ex3[:, hh, voff:QT],
                            start=(kb == 0), stop=(kb == last_kb))
                    pop_filler()
                # normalize via denominator rows 64:127
                for hh in range(2):
                    den = dsb.tile([DIM_HEAD, QT], F32, tag="den",
                                   name=f"den{gi}_{hh}")
                    if gi == 7:
                        nc.scalar.copy(den[:], po[hh][DIM_HEAD:P, :])
                    else:
                        nc.vector.tensor_copy(den[:], po[hh][DIM_HEAD:P, :])
                    rcp = dsb.tile([DIM_HEAD, QT], F32, tag="rcp",
                                   name=f"rcp{gi}_{hh}")
                    nc.vector.reciprocal_approx_fast(rcp[:], den[:])
                    nc.vector.tensor_tensor(
                        outT[pr][hh * DIM_HEAD:(hh + 1) * DIM_HEAD,
                                 qt * QT:(qt + 1) * QT],
                        po[hh][0:DIM_HEAD, :],
                        rcp[:],
                        ALU.mult)
                pop_filler()
                pop_filler()
                # schedule new work
                if gi == 0:
                    add_ln_units(2, 4)
                    add_vqk_units(2, 4)
                elif gi == 1:
                    add_op_units(0)
                elif gi == 2:
                    add_ln_units(3, 6)
                elif gi == 3:
                    add_vqk_units(3, 6)
                    add_op_units(4)
                elif gi == 5:
                    add_op_units(8)
                elif gi == 7:
                    add_op_units(12)
            _ps_state["startup"] = True   # attention done: psS slots free again
            while fillers:
                fillers.pop(0)[1]()

    nc.compile()
    return nc


def make_in_maps(x, ln_w, ln_b, w_qkv, w_out):
    x = np.asarray(x, np.float32)
    ln_w = np.asarray(ln_w, np.float32)
    ln_b = np.asarray(ln_b, np.float32)
    w_qkv = np.asarray(w_qkv, np.float32)
    w_out = np.asarray(w_out, np.float32)
    bf = ml_dtypes.bfloat16

    def chunk(w):
        # stage [128 p, 8 kb, M] with contraction index d = 128*kb + p
        return np.ascontiguousarray(
            w.reshape(KB, P, -1).transpose(1, 0, 2).astype(bf))

    tri = np.triu(np.ones((P, P), np.float32)).astype(bf)

    in_maps = []
    for c in range(N_CORES):
        b, g = c // 4, c % 4
        cols = np.arange(H_LOC * g * DIM_HEAD, H_LOC * (g + 1) * DIM_HEAD)
        wq_s = w_qkv[:, cols]
        wk_s = w_qkv[:, INNER + cols]
        wv_s = w_qkv[:, 2 * INNER + cols]
        wo_s = w_out[cols, :]
        in_maps.append({
            "x": np.ascontiguousarray(x[b]).astype(bf),
            "wq": chunk(ln_w[:, None] * wq_s * SCALE),
            "wk": chunk(ln_w[:, None] * wk_s),
            "wv": chunk(ln_w[:, None] * wv_s),
            "wo": np.ascontiguousarray(
                wo_s.reshape(2, 2, DIM_HEAD, DIM).transpose(1, 2, 0, 3)
                .reshape(P, 2, DIM).astype(bf)),
            "bq": np.ascontiguousarray(((ln_b @ wq_s) * SCALE).reshape(2, P).T),
            "bk": np.ascontiguousarray((ln_b @ wk_s).reshape(2, P).T),
            "bv": (ln_b @ wv_s).reshape(1, H_LOC, DIM_HEAD),
            "tri": tri,
        })
    return in_maps


_NC_CACHE = {}


def kernel(x, ln_w, ln_b, w_qkv, w_out):
    in_maps = make_in_maps(x, ln_w, ln_b, w_qkv, w_out)
    no_bias = not np.any(np.asarray(ln_b, np.float32))
    if no_bias not in _NC_CACHE:
        _NC_CACHE[no_bias] = build_nc(no_bias=no_bias)
    nc = _NC_CACHE[no_bias]
    res = run_bass_kernel_spmd(nc, in_maps, list(range(N_CORES))).results
    out = np.zeros((B, N, DIM), np.float32)
    for c in range(N_CORES):
        out[c // 4] += np.asarray(res[c]["out"], np.float32)
    return out


# revision 23
# speedup vs baseline: 1.2478x; 1.2478x over previous
"""Causal attention (LN -> QKV -> 16-head causal attn -> out-proj) on 8 TRN2 cores.

Sharding: core c = (batch b=c//4, head-group g=c%4); each core does its batch's
LayerNorm + 4 heads of QKV/attention/out-proj; host sums the 4 column-split
out-proj partials per batch.

Single fused pipeline per core:
  - x arrives bf16 per seq-quarter (st); LN fully on DVE (bn_stats + Quake
    rsqrt via int32 bit tricks) so ScalarE runs ONLY exp (ACT table loaded
    once, preloaded by a dummy activation).
  - xn transposed to xnT (chunk order d = 128*kb+p): PE identity-matmuls for
    st0 (startup latency; evacs split Scalar/DVE), DMA XBAR for st1-st3.
  - attention as 8 (pr, qt) groups of 512 queries x 2 heads; S^T for both
    heads in ONE [128,1024] psum tile (two 64-contraction matmuls, operands in
    per-head [64,*] Q/K tiles at partitions 0-63) -> ONE exp per (pr,qt,kb).
    The NEXT group's first S is prefetched inside the current group's last
    iteration so the exp stream never waits on group turnover.
  - V tiles carry 64 ones-columns so PV also lands 64 copies of the softmax
    denominator in psum rows 64:127: normalize = DVE copy + fast reciprocal +
    one multiply. Causal diagonal handled by a broadcast tri-mult on GpSimd
    (whose queue carries nothing else, so exp-waits cannot block other work).
  - QKV for st2/st3 and the out-projection are emitted as small "filler"
    units inside the attention kb loops to keep the PE busy during exp.
  PSUM: S [128,1024]x2bufs (4 banks) + psO 2x[128,512] (2) + general-purpose
  [128,1024] (2) = 8 banks exactly.
"""

import numpy as np
import ml_dtypes

import concourse.bass as bass
import concourse.mybir as mybir
import concourse.tile as tile
from concourse import bacc
from concourse.bass_utils import run_bass_kernel_spmd
from concourse.masks import make_identity

B, N, DIM, HEADS, DIM_HEAD = 2, 2048, 1024, 16, 64
INNER = HEADS * DIM_HEAD
H_LOC = 4
N_CORES = 8
P = 128
NB = N // P
KB = DIM // P
QT = 512
SCALE = DIM_HEAD ** -0.5
LN_EPS = 1e-5
MAGIC = 0x5F3759DF

F32 = mybir.dt.float32
BF16 = mybir.dt.bfloat16
I32 = mybir.dt.int32
AF = mybir.ActivationFunctionType
ALU = mybir.AluOpType


def build_nc(no_bias=False):
    from contextlib import ExitStack

    nc = bacc.Bacc(None, target_bir_lowering=False, debug=False)

    x_d = nc.dram_tensor("x", [N, DIM], BF16, kind="ExternalInput")
    wq_d = nc.dram_tensor("wq", [P, KB, 2 * P], BF16, kind="ExternalInput")
    wk_d = nc.dram_tensor("wk", [P, KB, 2 * P], BF16, kind="ExternalInput")
    wv_d = nc.dram_tensor("wv", [P, KB, 2 * P], BF16, kind="ExternalInput")
    wo_d = nc.dram_tensor("wo", [P, 2, DIM], BF16, kind="ExternalInput")
    bq_d = nc.dram_tensor("bq", [P, 2], F32, kind="ExternalInput")
    bk_d = nc.dram_tensor("bk", [P, 2], F32, kind="ExternalInput")
    bv_d = nc.dram_tensor("bv", [1, H_LOC, DIM_HEAD], F32, kind="ExternalInput")
    tri_d = nc.dram_tensor("tri", [P, P], BF16, kind="ExternalInput")
    out_d = nc.dram_tensor("out", [N, DIM], BF16, kind="ExternalOutput")

    with tile.TileContext(nc) as tc:
        ctx = ExitStack()
        with ctx:
            const = ctx.enter_context(tc.tile_pool(name="const", bufs=1))
            persist = ctx.enter_context(tc.tile_pool(name="persist", bufs=1))
            xnpool = ctx.enter_context(tc.tile_pool(name="xnpool", bufs=4))
            statp = ctx.enter_context(tc.tile_pool(name="statp", bufs=2))
            expp = ctx.enter_context(tc.tile_pool(name="expp", bufs=4))
            dsb = ctx.enter_context(tc.tile_pool(name="dsb", bufs=2))
            stage = ctx.enter_context(tc.tile_pool(name="stage", bufs=3))
            psS = ctx.enter_context(tc.tile_pool(name="psS", bufs=2, space="PSUM"))
            psO = ctx.enter_context(tc.tile_pool(name="psO", bufs=1, space="PSUM"))
            psG = ctx.enter_context(tc.tile_pool(name="psG", bufs=1, space="PSUM"))

            # exp ACT-table preload
            dmy_f = const.tile([P, 1], F32, tag="dmy_f", name="dmy_f")
            nc.vector.memset(dmy_f, 0.0)
            dmy_b = const.tile([P, 1], BF16, tag="dmy_b", name="dmy_b")
            nc.scalar.activation(dmy_b[:], dmy_f[:], AF.Exp)
            ident = const.tile([P, P], BF16, tag="ident", name="ident")
            make_identity(nc, ident)

            # ---- input DMAs: x(st0) first, weights interleaved ----
            x_st = [persist.tile([P, 4, DIM], BF16, tag=f"xs{st}", name=f"xs{st}")
                    for st in range(4)]
            for j in range(4):
                nc.sync.dma_start(x_st[0][:, j, :], x_d[j * P:(j + 1) * P, :])
            wq_sb = persist.tile([P, KB, 2 * P], BF16, tag="wq", name="wq_sb")
            nc.sync.dma_start(wq_sb[:], wq_d[:])
            wk_sb = persist.tile([P, KB, 2 * P], BF16, tag="wk", name="wk_sb")
            nc.sync.dma_start(wk_sb[:], wk_d[:])
            nc.sync.dma_start(
                x_st[1][:], x_d[512:1024, :].rearrange("(s p) d -> p s d", p=P))
            wv_sb = persist.tile([P, KB, 2 * P], BF16, tag="wv", name="wv_sb")
            nc.sync.dma_start(wv_sb[:], wv_d[:])
            nc.sync.dma_start(
                x_st[2][:], x_d[1024:1536, :].rearrange("(s p) d -> p s d", p=P))
            nc.sync.dma_start(
                x_st[3][:], x_d[1536:2048, :].rearrange("(s p) d -> p s d", p=P))
            wo_sb = persist.tile([P, 2, DIM], BF16, tag="wo", name="wo_sb")
            nc.sync.dma_start(wo_sb[:], wo_d[:])
            bq_sb = const.tile([P, 2], F32, tag="bq", name="bq_sb")
            nc.sync.dma_start(bq_sb[:], bq_d[:])
            bk_sb = const.tile([P, 2], F32, tag="bk", name="bk_sb")
            nc.sync.dma_start(bk_sb[:], bk_d[:])
            bv_sb = const.tile([P, H_LOC, DIM_HEAD], F32, tag="bv", name="bv_sb")
            nc.sync.dma_start(bv_sb[:], bv_d[:].to_broadcast((P, H_LOC, DIM_HEAD)))
            tri_t = const.tile([P, P], BF16, tag="tri", name="tri_t")
            nc.sync.dma_start(tri_t[:], tri_d[:])
            magic_t = const.tile([P, 4], I32, tag="magic", name="magic_t")
            nc.vector.memset(magic_t, MAGIC)

            xnT = [persist.tile([P, KB, 4 * P], BF16, tag=f"xnT{st}",
                                name=f"xnT{st}") for st in range(4)]
            QTh = [[persist.tile([DIM_HEAD, N], BF16, tag=f"qt{pr}{hh}",
                                 name=f"qt{pr}{hh}") for hh in range(2)]
                   for pr in range(2)]
            KTh = [[persist.tile([DIM_HEAD, N], BF16, tag=f"kt{pr}{hh}",
                                 name=f"kt{pr}{hh}") for hh in range(2)]
                   for pr in range(2)]
            Vt = persist.tile([P, NB, H_LOC, P], BF16, tag="v", name="Vt")
            nc.gpsimd.memset(Vt[:], 1.0)
            outT = [persist.tile([P, N], BF16, tag=f"outT{pr}", name=f"outT{pr}")
                    for pr in range(2)]

            # ---- LN / QKV building blocks ----
            # psum allocator for accumulation chains: during startup rotate
            # across psG + the two (still unused) psS slots so transpose/V/QK
            # chains overlap their evacuations; attention-era fillers use psG.
            _ps_state = {"rr": 0, "startup": True}

            def mkps(name):
                if _ps_state["startup"]:
                    _ps_state["rr"] = (_ps_state["rr"] + 1) % 3
                    if _ps_state["rr"]:
                        return psS.tile([P, 2 * QT], F32, tag="sps",
                                        name=f"ps_{name}")
                return psG.tile([P, KB * P], F32, tag="gp", name=f"ps_{name}")

            stp_t = {}

            def u_stats(st, j):
                if st not in stp_t:
                    stp_t[st] = statp.tile([P, 4, 2], F32, tag="stp",
                                           name=f"stp{st}")
                st6 = statp.tile([P, 2, 6], F32, tag="st6", name=f"st6_{st}_{j}")
                x3 = x_st[st][:, j, :].rearrange("p (a f) -> p a f", a=2)
                for a in range(2):
                    nc.vector.bn_stats(st6[:, a, :], x3[:, a, :])
                nc.vector.bn_aggr(stp_t[st][:, j, :], st6[:])

            rs_t = {}

            def u_rsqrt(st, j0, n):
                stp = stp_t[st]
                veps = statp.tile([P, 4], F32, tag="veps", name=f"ve{st}{j0}")
                nc.vector.tensor_scalar_add(
                    veps[:, 0:n], stp[:, j0:j0 + n, 1], LN_EPS)
                iv = statp.tile([P, 4], I32, tag="iv", name=f"iv{st}{j0}")
                nc.vector.tensor_scalar(
                    iv[:, 0:n], veps[:, 0:n].bitcast(I32), 1, None,
                    ALU.logical_shift_right)
                y0i = statp.tile([P, 4], I32, tag="y0i", name=f"y0{st}{j0}")
                nc.vector.tensor_tensor(
                    y0i[:, 0:n], magic_t[:, 0:n], iv[:, 0:n], ALU.subtract)
                t1 = statp.tile([P, 4], F32, tag="t1", name=f"t1{st}{j0}")
                if (st, 0) not in rs_t:
                    rs_t[(st, 0)] = statp.tile([P, 4], F32, tag="rstd",
                                               name=f"rstd{st}")
                    rs_t[(st, 1)] = statp.tile([P, 4], F32, tag="nmrs",
                                               name=f"nmrs{st}")
                rstd = rs_t[(st, 0)][:, j0:j0 + n]
                nmrs = rs_t[(st, 1)][:, j0:j0 + n]
                for it in range(2):
                    src = y0i[:, 0:n].bitcast(F32) if it == 0 else rstd
                    nc.vector.tensor_tensor(t1[:, 0:n], src, src, ALU.mult)
                    nc.vector.tensor_tensor(
                        t1[:, 0:n], t1[:, 0:n], veps[:, 0:n], ALU.mult)
                    nc.vector.tensor_scalar(
                        t1[:, 0:n], t1[:, 0:n], -0.5, 1.5, ALU.mult, ALU.add)
                    nc.vector.tensor_tensor(rstd, src, t1[:, 0:n], ALU.mult)
                nc.vector.tensor_tensor(
                    nmrs, stp[:, j0:j0 + n, 0], rstd, ALU.mult)
                nc.vector.tensor_scalar_mul(nmrs, nmrs, -1.0)

            def u_xnT(st, j):
                sb = st * 4 + j
                xn = xnpool.tile([P, DIM], BF16, tag="xn", name=f"xn{sb}")
                nc.vector.tensor_scalar(
                    xn[:], x_st[st][:, j, :],
                    rs_t[(st, 0)][:, j:j + 1], rs_t[(st, 1)][:, j:j + 1],
                    ALU.mult, ALU.add)
                if st == 0:
                    gp = mkps(f"gpt{sb}")
                    for kb in range(KB):
                        nc.tensor.matmul(
                            gp[:, kb * P:(kb + 1) * P],
                            xn[:, kb * P:(kb + 1) * P],
                            ident[:],
                            start=True, stop=True)
                    dst = xnT[st][:, :, j * P:(j + 1) * P]
                    srcv = gp[:].rearrange("p (kb s) -> p kb s", kb=KB)
                    if j % 2 == 0:
                        nc.scalar.copy(dst, srcv)
                    else:
                        nc.vector.tensor_copy(dst, srcv)
                else:
                    nc.sync.dma_start(
                        xnT[st][:, :, j * P:(j + 1) * P], xn[:], transpose=True)

            def u_v(st, j):
                sb = st * 4 + j
                gp = mkps(f"gpv{sb}")
                for kb in range(KB):
                    nc.tensor.matmul(
                        gp[:, 0:2 * P],
                        xnT[st][:, kb, j * P:(j + 1) * P],
                        wv_sb[:, kb, :],
                        start=(kb == 0), stop=(kb == KB - 1))
                nc.vector.tensor_tensor(
                    Vt[:, sb, :, 0:DIM_HEAD],
                    gp[:, 0:2 * P].rearrange("p (h d) -> p h d", h=H_LOC),
                    bv_sb[:],
                    ALU.add)

            def u_qk(st, wt, pr):
                w_sb, bias, dst = (
                    (wq_sb, bq_sb, QTh) if wt == 0 else (wk_sb, bk_sb, KTh))
                gp = mkps(f"gpqk{st}{wt}{pr}")
                for kb in range(KB):
                    nc.tensor.matmul(
                        gp[:, 0:512],
                        w_sb[:, kb, pr * P:(pr + 1) * P],
                        xnT[st][:, kb, :],
                        start=(kb == 0), stop=(kb == KB - 1))
                for hh in range(2):
                    d_ap = dst[pr][hh][:, st * 512:(st + 1) * 512]
                    s_ap = gp[hh * DIM_HEAD:(hh + 1) * DIM_HEAD, 0:512]
                    if no_bias and st == 1:
                        # ScalarE COPY needs no ACT table; its queue has slack
                        # in the small-exp era where st1 QKV must land, while
                        # DVE is congested there.
                        nc.scalar.copy(d_ap, s_ap)
                    else:
                        nc.vector.tensor_scalar_add(
                            d_ap, s_ap,
                            bias[hh * DIM_HEAD:(hh + 1) * DIM_HEAD, pr:pr + 1])

            def u_outproj(qb):
                gp = mkps(f"gpo{qb}")
                for nt in range(2):
                    for pr in range(2):
                        nc.tensor.matmul(
                            gp[:, nt * 512:(nt + 1) * 512],
                            outT[pr][:, qb * P:(qb + 1) * P],
                            wo_sb[:, pr, nt * 512:(nt + 1) * 512],
                            start=(pr == 0), stop=(pr == 1))
                so = stage.tile([P, DIM], BF16, tag="so", name=f"so{qb}")
                if qb >= 12 and qb % 2:
                    nc.scalar.copy(so[:], gp[:])
                else:
                    nc.vector.tensor_copy(so[:], gp[:])
                nc.sync.dma_start(out_d[qb * P:(qb + 1) * P, :], so[:])

            # ---- straight phase A for st0 (latency-tuned) and st1-LN ----
            for pair in range(2):
                for j in (2 * pair, 2 * pair + 1):
                    u_stats(0, j)
                u_rsqrt(0, 2 * pair, 2)
                for j in (2 * pair, 2 * pair + 1):
                    u_xnT(0, j)
            for wt in range(2):
                for pr in range(2):
                    u_qk(0, wt, pr)
            u_v(0, 0)
            for j in range(4):
                u_stats(1, j)
            u_rsqrt(1, 0, 4)
            for j in range(4):
                u_xnT(1, j)
            _ps_state["startup"] = False

            # ---- fillers ----
            fillers = []

            def pop_filler():
                if fillers:
                    fillers.pop(0)[1]()

            def drain_required(gi):
                while fillers and fillers[0][0] <= gi:
                    fillers.pop(0)[1]()

            def add_ln_units(st, deadline):
                for j in range(4):
                    fillers.append(
                        (deadline, (lambda s, j: lambda: u_stats(s, j))(st, j)))
                fillers.append(
                    (deadline, (lambda s: lambda: u_rsqrt(s, 0, 4))(st)))
                for j in range(4):
                    fillers.append(
                        (deadline, (lambda s, j: lambda: u_xnT(s, j))(st, j)))

            def add_vqk_units(st, deadline):
                for j in range(4):
                    fillers.append(
                        (deadline, (lambda s, j: lambda: u_v(s, j))(st, j)))
                for wt in range(2):
                    for pr in range(2):
                        fillers.append(
                            (deadline,
                             (lambda s, w, p_: lambda: u_qk(s, w, p_))(st, wt, pr)))

            def add_op_units(q0):
                for qb in range(q0, q0 + 4):
                    fillers.append((99, (lambda q: lambda: u_outproj(q))(qb)))

            for j in range(1, 4):
                fillers.append((0, (lambda jj: lambda: u_v(0, jj))(j)))
            for j in range(4):
                fillers.append((2, (lambda jj: lambda: u_v(1, jj))(j)))
            for wt in range(2):
                for pr in range(2):
                    fillers.append(
                        (2, (lambda w, p_: lambda: u_qk(1, w, p_))(wt, pr)))

            # ---- fused attention pipeline ----
            groups = [(pr, qt) for qt in range(4) for pr in range(2)]

            def s_mm(pr, qt, kb):
                voff = max(0, (kb - 4 * qt) * P)
                sps = psS.tile([P, 2 * QT], F32, tag="sps",
                               name=f"sps{pr}_{qt}_{kb}")
                for hh in range(2):
                    nc.tensor.matmul(
                        sps[:, hh * QT + voff:(hh + 1) * QT],
                        KTh[pr][hh][:, kb * P:(kb + 1) * P],
                        QTh[pr][hh][:, qt * QT + voff:(qt + 1) * QT],
                        start=True, stop=True)
                return sps, voff

            sps_next = s_mm(0, 0, 0)
            for gi, (pr, qt) in enumerate(groups):
                last_kb = 4 * qt + 3
                po = [psO.tile([P, QT], F32, tag=f"po{hh}", name=f"po{hh}_{gi}")
                      for hh in range(2)]
                for kb in range(last_kb + 1):
                    sps, voff = sps_next
                    ex = expp.tile([P, 2 * QT], BF16, tag="ex",
                                   name=f"ex{gi}_{kb}")
                    ex3 = ex[:].rearrange("p (h q) -> p h q", h=2)
                    sp3 = sps[:].rearrange("p (h q) -> p h q", h=2)
                    nc.scalar.activation(
                        ex3[:, :, voff:QT], sp3[:, :, voff:QT], AF.Exp)
                    if kb < last_kb:
                        sps_next = s_mm(pr, qt, kb + 1)
                    elif gi + 1 < len(groups):
                        drain_required(gi + 1)
                        npr, nqt = groups[gi + 1]
                        sps_next = s_mm(npr, nqt, 0)
                    if kb >= 4 * qt:
                        tri_b = tri_t[:].rearrange(
                            "p (o q) -> p o q", o=1).to_broadcast((P, 2, P))
                        nc.gpsimd.tensor_tensor(
                            ex3[:, :, voff:voff + P],
                            ex3[:, :, voff:voff + P],
                            tri_b,
                            ALU.mult)
                    for hh in range(2):
                        nc.tensor.matmul(
                            po[hh][:, voff:QT],
                            Vt[:, kb, 2 * pr + hh, :],
                            ex3[:, hh, voff:QT],
                            start=(kb == 0), stop=(kb == last_kb))
                    pop_filler()
                # normalize via denominator rows 64:127
                for hh in range(2):
                    den = dsb.tile([DIM_HEAD, QT], F32, tag="den",
                                   name=f"den{gi}_{hh}")
                    if gi == 7:
                        nc.scalar.copy(den[:], po[hh][DIM_HEAD:P, :])
                    else:
                        nc.vector.tensor_copy(den[:], po[hh][DIM_HEAD:P, :])
                    rcp = dsb.tile([DIM_HEAD, QT], F32, tag="rcp",
                                   name=f"rcp{gi}_{hh}")
                    nc.vector.reciprocal_approx_fast(rcp[:], den[:])
                    nc.vector.tensor_tensor(
                        outT[pr][hh * DIM_HEAD:(hh + 1) * DIM_HEAD,
                                 qt * QT:(qt + 1) * QT],
                        po[hh][0:DIM_HEAD, :],
                        rcp[:],
                        ALU.mult)
                pop_filler()
                pop_filler()
                # schedule new work
                if gi == 0:
                    add_ln_units(2, 4)
                    add_vqk_units(2, 4)
                elif gi == 1:
                    add_op_units(0)
                elif gi == 2:
                    add_ln_units(3, 6)
                elif gi == 3:
                    add_vqk_units(3, 6)
                    add_op_units(4)
                elif gi == 5:
                    add_op_units(8)
                elif gi == 7:
                    add_op_units(12)
            _ps_state["startup"] = True   # attention done: psS slots free again
            while fillers:
                fillers.pop(0)[1]()

    nc.compile()
    return nc


def make_in_maps(x, ln_w, ln_b, w_qkv, w_out):
    x = np.asarray(x, np.float32)
    ln_w = np.asarray(ln_w, np.float32)
    ln_b = np.asarray(ln_b, np.float32)
    w_qkv = np.asarray(w_qkv, np.float32)
    w_out = np.asarray(w_out, np.float32)
    bf = ml_dtypes.bfloat16

    def chunk(w):
        # stage [128 p, 8 kb, M] with contraction index d = 128*kb + p
        return np.ascontiguousarray(
            w.reshape(KB, P, -1).transpose(1, 0, 2).astype(bf))

    tri = np.triu(np.ones((P, P), np.float32)).astype(bf)

    in_maps = []
    for c in range(N_CORES):
        b, g = c // 4, c % 4
        cols = np.arange(H_LOC * g * DIM_HEAD, H_LOC * (g + 1) * DIM_HEAD)
        wq_s = w_qkv[:, cols]
        wk_s = w_qkv[:, INNER + cols]
        wv_s = w_qkv[:, 2 * INNER + cols]
        wo_s = w_out[cols, :]
        in_maps.append({
            "x": np.ascontiguousarray(x[b]).astype(bf),
            "wq": chunk(ln_w[:, None] * wq_s * SCALE),
            "wk": chunk(ln_w[:, None] * wk_s),
            "wv": chunk(ln_w[:, None] * wv_s),
            "wo": np.ascontiguousarray(
                wo_s.reshape(2, 2, DIM_HEAD, DIM).transpose(1, 2, 0, 3)
                .reshape(P, 2, DIM).astype(bf)),
            "bq": np.ascontiguousarray(((ln_b @ wq_s) * SCALE).reshape(2, P).T),
            "bk": np.ascontiguousarray((ln_b @ wk_s).reshape(2, P).T),
            "bv": (ln_b @ wv_s).reshape(1, H_LOC, DIM_HEAD),
            "tri": tri,
        })
    return in_maps


_NC_CACHE = {}


def kernel(x, ln_w, ln_b, w_qkv, w_out):
    in_maps = make_in_maps(x, ln_w, ln_b, w_qkv, w_out)
    no_bias = not np.any(np.asarray(ln_b, np.float32))
    if no_bias not in _NC_CACHE:
        _NC_CACHE[no_bias] = build_nc(no_bias=no_bias)
    nc = _NC_CACHE[no_bias]
    res = run_bass_kernel_spmd(nc, in_maps, list(range(N_CORES))).results
    out = np.zeros((B, N, DIM), np.float32)
    for c in range(N_CORES):
        out[c // 4] += np.asarray(res[c]["out"], np.float32)
    return out
